# revision 6
# baseline (speedup 1.0000x reference)
"""Trainium2 Bass kernel for nn_BaseBranch_6811818132502 (dense_cnn).

Strategy:
 - Host-side (exact, verified vs reference): fold the channel-permutation
   einsum into conv1 weights, fold the rot90/rot-back pairs into spatially
   rotated 3x3 kernels, fold BN+conv-bias into per-channel scale/bias, and
   replace the pad-20 odd passes with pad-4 (receptive field is 5, implicit
   conv padding covers the outermost ring).  The whole module becomes:
   8 passes of [conv3x3(d=1) -> conv3x3(d=2) -> conv3x3(d=2)], each with
   fused scale/bias(+relu), then a global max over 8*64 channels, sigmoid,
   clip.
 - Device-side: data-parallel over batch (1 image per core, 8 cores).
   Same-parity anchor passes are processed in PAIRS so the 128-wide PE
   array is fully used despite only 64 output channels per pass.  bf16
   weights/activations (fp32 PSUM accumulation) -- fp32r occupies two PE
   columns per output channel, so only bf16 can use the array's full
   width:
     * conv1 of a pair shares its rhs (the same padded image with a
       1-row-shifted copy in partitions 64:128 for kh-pairing), so one
       dense K=128 M=128 matmul per tap computes BOTH passes -- 6 matmuls
       per chunk for the pair.
     * conv2/conv3 keep activations pair-packed ([pass_a | pass_b] on
       partitions 0:64 / 64:128) and run each 3x3 tap as two concurrent
       disjoint-quadrant 64x64 tiles (tile positions (0,0)/(64,64)) --
       9 matmul slots per chunk for both passes.
   ScalarE evicts PSUM with fused scale/bias/ReLU ([128,n] per chunk =
   both passes at once); conv3 evicts with scale/bias then DVE keeps the
   running channel max in a pair-packed fp32 acc; a PE-transpose +
   DVE-max tree reduces over all 128 packed channels at the end.
"""
import sys
import os
import math

for _p in ("/opt/trn_rl_repo", "/root/.axon_site/_ro/trn_rl_repo"):
    if os.path.isdir(_p) and _p not in sys.path:
        sys.path.insert(0, _p)

import numpy as np
import ml_dtypes

import concourse.bass as bass
import concourse.mybir as mybir
import concourse.tile as tile
from concourse import bacc, masks
from concourse.bass_utils import run_bass_kernel_spmd
from contextlib import ExitStack

F32 = mybir.dt.float32
BF16 = mybir.dt.bfloat16
NP_BF16 = np.dtype(ml_dtypes.bfloat16)

BN_EPS = 1e-5
C = 64            # channels
H = W = 96        # map size
B = 8             # batch == n_cores
PAD = 4           # explicit pad for odd passes (exact; receptive field 5)
R = 4             # output rows per PSUM chunk

XO_S = H + 2 * PAD + 2      # 106: x + pad4 + conv1 halo 1
Y1_S = H + 2 * PAD          # 104: conv1 odd output domain (even uses interior)
Y2_S = H + 4                # 100: conv2 output domain (coords -2..97)

# geometry per (layer, parity): dilation, rhs row/col base offset,
# out rows/cols, output write offset into the destination buffer
GEOM = {
    (0, 0): dict(d=1, off=4, oh=96,  ow=96,  woff=4),   # conv1 even
    (0, 1): dict(d=1, off=0, oh=104, ow=104, woff=0),   # conv1 odd
    (1, 0): dict(d=2, off=2, oh=96,  ow=96,  woff=2),   # conv2 even
    (1, 1): dict(d=2, off=0, oh=100, ow=100, woff=0),   # conv2 odd
    (2, 0): dict(d=2, off=0, oh=96,  ow=96,  woff=None),  # conv3 -> ACC
    (2, 1): dict(d=2, off=0, oh=96,  ow=96,  woff=None),
}
# evens first: odd passes overwrite the zero borders of y1/y2 that even
# passes rely on for implicit padding
PAIR_SEQ = [(0, 2), (4, 6), (1, 3), (5, 7)]
C1_COL = 0               # conv1 tap j at j*128 (A cols +0:64, B cols +64:128)
C2_COL = 6 * 128         # conv2 tap t at C2_COL + t*64 (A rows 0:64, B rows 64:128)
C3_COL = C2_COL + 9 * 64
W_BLOB_COLS = C3_COL + 9 * 64   # 1920

_PROGRAM_CACHE = {}
TRACE = False
LAST_EXEC_NS = None


def _build_program():
    nc = bacc.Bacc("TRN2", target_bir_lowering=False, debug=False, num_devices=B)
    x_in = nc.dram_tensor("x_in", [C, H, W], BF16, kind="ExternalInput")
    w_in = nc.dram_tensor("w_in", [4, 128, W_BLOB_COLS], BF16, kind="ExternalInput")
    sc_in = nc.dram_tensor("sc_in", [128, 3], F32, kind="ExternalInput")
    bi_in = nc.dram_tensor("bi_in", [128, 3], F32, kind="ExternalInput")
    o_out = nc.dram_tensor("o_out", [1, H * W], F32, kind="ExternalOutput")

    with ExitStack() as ctx:
        tc = ctx.enter_context(tile.TileContext(nc))
        bigpool = ctx.enter_context(tc.tile_pool(name="big", bufs=1))
        wpool = ctx.enter_context(tc.tile_pool(name="wts", bufs=2))
        evpool = ctx.enter_context(tc.tile_pool(name="ev", bufs=4))
        psum = ctx.enter_context(tc.tile_pool(name="ps", bufs=6, space="PSUM"))
        tpsum = ctx.enter_context(tc.tile_pool(name="tps", bufs=2, space="PSUM"))

        xo = bigpool.tile([128, XO_S, XO_S], BF16)
        y1 = bigpool.tile([128, Y1_S, Y1_S], BF16)
        y2 = bigpool.tile([128, Y2_S, Y2_S], BF16)
        acc = bigpool.tile([128, H * W], F32)
        sct = bigpool.tile([128, 3], F32)
        bit = bigpool.tile([128, 3], F32)

        # xo: lower half = x with a PAD+1 zero ring; upper half = the same
        # image shifted up one row (for conv1 kh-pairing).  Both halves are
        # DMAed straight from DRAM in parallel; DVE zeroes the halo rings.
        P1 = PAD + 1
        lo, hi = xo[0:C], xo[C:128]
        nc.vector.memset(lo[:, 0:P1, :], 0.0)
        nc.vector.memset(lo[:, XO_S - P1:XO_S, :], 0.0)
        nc.vector.memset(lo[:, P1:XO_S - P1, 0:P1], 0.0)
        nc.vector.memset(lo[:, P1:XO_S - P1, XO_S - P1:XO_S], 0.0)
        nc.vector.memset(hi[:, 0:P1 - 1, :], 0.0)
        nc.vector.memset(hi[:, XO_S - P1 - 1:XO_S, :], 0.0)
        nc.vector.memset(hi[:, P1 - 1:XO_S - P1 - 1, 0:P1], 0.0)
        nc.vector.memset(hi[:, P1 - 1:XO_S - P1 - 1, XO_S - P1:XO_S], 0.0)
        nc.gpsimd.dma_start(out=lo[:, P1:P1 + H, P1:P1 + W], in_=x_in[:, :, :])
        nc.sync.dma_start(out=hi[:, P1 - 1:P1 - 1 + H, P1:P1 + W], in_=x_in[:, :, :])
        nc.gpsimd.dma_start(out=sct, in_=sc_in[:, :])
        nc.gpsimd.dma_start(out=bit, in_=bi_in[:, :])
        nc.gpsimd.memset(y1, 0.0)
        nc.gpsimd.memset(y2, 0.0)
        nc.gpsimd.memset(acc, 0.0)

        bufs = [xo, y1, y2, None]

        for pi, (pa, pb) in enumerate(PAIR_SEQ):
            parity = pa % 2
            wt = wpool.tile([128, W_BLOB_COLS], BF16, tag="wt")
            nc.gpsimd.dma_start(out=wt, in_=w_in[pi, :, :])

            for l in range(3):
                g = GEOM[(l, parity)]
                d, off, oh, ow, woff = g["d"], g["off"], g["oh"], g["ow"], g["woff"]
                src = bufs[l]
                dst = bufs[l + 1]
                h0 = 0
                while h0 < oh:
                    rr = min(R, oh - h0)
                    n = rr * ow
                    pt = psum.tile([128, 512], F32, tag="pt")
                    if l == 0:
                        # conv1: both passes in one dense K=128 M=128 matmul
                        for j in range(6):
                            kw = j % 3
                            kh0 = 0 if j < 3 else 2
                            rbase = h0 + kh0 * d + off
                            cbase = kw * d + off
                            rhs = src[0:128, rbase:rbase + rr, cbase:cbase + ow]
                            nc.tensor.matmul(pt[:, 0:n],
                                             wt[:, j * 128:(j + 1) * 128], rhs,
                                             start=(j == 0), stop=(j == 5))
                    else:
                        # conv2/conv3: per-tap, two concurrent 64x64 quadrant
                        # tiles (pass A rows 0:64, pass B rows 64:128)
                        base_col = C2_COL if l == 1 else C3_COL
                        for t in range(9):
                            kh, kw = divmod(t, 3)
                            rbase = h0 + kh * d + off
                            cbase = kw * d + off
                            rhsA = src[0:64, rbase:rbase + rr, cbase:cbase + ow]
                            rhsB = src[64:128, rbase:rbase + rr, cbase:cbase + ow]
                            c0 = base_col + t * 64
                            lA = wt[0:64, c0:c0 + 64]
                            lB = wt[64:128, c0:c0 + 64]
                            nc.tensor.matmul(pt[0:64, 0:n], lA, rhsA,
                                             start=(t == 0), stop=(t == 8))
                            nc.tensor.matmul(pt[64:128, 0:n], lB, rhsB,
                                             start=(t == 0), stop=(t == 8))
                    if l < 2:
                        # evict both passes at once with scale/bias + relu
                        a = h0 + woff
                        nc.scalar.activation(
                            out=dst[0:128, a:a + rr, woff:woff + ow],
                            in_=pt[:, 0:n].rearrange("p (r c) -> p r c", r=rr),
                            func=mybir.ActivationFunctionType.Relu,
                            bias=bit[:, l:l + 1], scale=sct[:, l:l + 1])
                    else:
                        # conv3: scale/bias then running channel max; acc is
                        # pair-packed [128, H*W] -- cross-half max happens in
                        # the final channel reduction
                        tmp = evpool.tile([128, 512], F32, tag="ev")
                        nc.scalar.activation(
                            out=tmp[:, 0:n], in_=pt[:, 0:n],
                            func=mybir.ActivationFunctionType.Identity,
                            bias=bit[:, 2:3], scale=sct[:, 2:3])
                        nc.vector.tensor_max(
                            acc[:, h0 * W:h0 * W + n],
                            acc[:, h0 * W:h0 * W + n],
                            tmp[:, 0:n])
                    h0 += rr

        # channel-max reduction: PE-transpose 128-col blocks of acc into PSUM
        # ([128, 128] -> [128, 128]) and reduce over the free dim (both pair
        # halves' channels) on DVE.  72 blocks in 18 groups of 4; each
        # group's transposes start as soon as the conv3 maxes land.
        ident = bigpool.tile([128, 128], F32)
        masks.make_identity(nc, ident)
        red = bigpool.tile([128, 72], F32)
        NB = (H * W) // 128  # 72 blocks
        for g in range(NB // 4):
            ps = tpsum.tile([128, 512], F32, tag="tp")
            for b in range(4):
                blk = g * 4 + b
                nc.tensor.transpose(ps[:, b * 128:(b + 1) * 128],
                                    acc[:, blk * 128:(blk + 1) * 128],
                                    ident[:, :])
            nc.vector.tensor_reduce(out=red[:, g * 4:(g + 1) * 4],
                                    in_=ps.rearrange("p (b c) -> p b c", b=4),
                                    axis=mybir.AxisListType.X,
                                    op=mybir.AluOpType.max)
        ps2 = tpsum.tile([128, 512], F32, tag="tp")
        nc.tensor.transpose(ps2[0:72, 0:128], red[:, :], ident[:, :])
        rsb = bigpool.tile([72, 128], F32)
        nc.scalar.activation(out=rsb, in_=ps2[0:72, 0:128],
                             func=mybir.ActivationFunctionType.Sigmoid)
        nc.vector.tensor_scalar(rsb, rsb, 1e-4, 1.0 - 1e-4,
                                mybir.AluOpType.max, mybir.AluOpType.min)
        nc.sync.dma_start(
            out=o_out.ap().rearrange("a (c r) -> a c r", r=128), in_=rsb)
    nc.compile()
    return nc


def _fold_weights(perms, dcn_w, dcn_b, conv2_w, conv2_b, conv3_w, conv3_b,
                  bn_gamma, bn_beta, bn_mean, bn_var):
    """Fold rotations/permutation/BN on the host. Returns (w_blob, scales, biases)."""
    scales = np.empty((128, 3), np.float32)
    biases = np.empty((128, 3), np.float32)
    conv_bs = [dcn_b, conv2_b, conv3_b]
    for l in range(3):
        s = bn_gamma[l] / np.sqrt(bn_var[l] + BN_EPS)
        bl = bn_beta[l] - bn_mean[l] * s + conv_bs[l] * s
        scales[0:C, l] = s
        scales[C:128, l] = s
        biases[0:C, l] = bl
        biases[C:128, l] = bl

    w_blob = np.zeros((4, 128, W_BLOB_COLS), np.float32)
    base_ws = [dcn_w, conv2_w, conv3_w]
    for pi, pair in enumerate(PAIR_SEQ):
        for half, p in enumerate(pair):
            k = p % 4
            # conv1: rotation + channel permutation folded
            w1 = np.rot90(base_ws[0], k=-k, axes=(-2, -1))
            w1 = np.einsum('omhw,mj->ojhw', w1, perms[p], optimize=True)
            for j in range(6):
                kw = j % 3
                col = j * 128 + half * 64
                if j < 3:
                    w_blob[pi, 0:C, col:col + C] = w1[:, :, 0, kw].T
                    w_blob[pi, C:128, col:col + C] = w1[:, :, 1, kw].T
                else:
                    w_blob[pi, 0:C, col:col + C] = w1[:, :, 2, kw].T
            rows = slice(half * C, half * C + C)
            for l, base_col in ((1, C2_COL), (2, C3_COL)):
                wl = np.rot90(base_ws[l], k=-k, axes=(-2, -1))
                for t in range(9):
                    kh, kw = divmod(t, 3)
                    w_blob[pi, rows, base_col + t * 64:base_col + t * 64 + C] = \
                        wl[:, :, kh, kw].T
    return w_blob, scales, biases


def kernel(x, perms, dcn_w, dcn_b, conv2_w, conv2_b, conv3_w, conv3_b,
           bn_gamma, bn_beta, bn_mean, bn_var):
    global LAST_EXEC_NS
    x = np.ascontiguousarray(np.asarray(x, np.float32))
    args = [np.asarray(a, np.float32) for a in
            (perms, dcn_w, dcn_b, conv2_w, conv2_b, conv3_w, conv3_b,
             bn_gamma, bn_beta, bn_mean, bn_var)]
    w_blob, scales, biases = _fold_weights(*args)
    w_blob = w_blob.astype(NP_BF16)
    x_bf = x.astype(NP_BF16)

    if "prog" not in _PROGRAM_CACHE:
        _PROGRAM_CACHE["prog"] = _build_program()
    nc = _PROGRAM_CACHE["prog"]

    in_maps = [{
        "x_in": np.ascontiguousarray(x_bf[b]),
        "w_in": w_blob,
        "sc_in": scales,
        "bi_in": biases,
    } for b in range(B)]

    r = run_bass_kernel_spmd(nc, in_maps, core_ids=list(range(B)), trace=TRACE)
    LAST_EXEC_NS = r.exec_time_ns
    out = np.stack([r.results[b]["o_out"].reshape(1, H, W) for b in range(B)])
    return out.astype(np.float32)
